# revision 22
# baseline (speedup 1.0000x reference)
"""BitNet MLP (SwiGLU, ternary weights) on 8 Trainium2 NeuronCores.

Strategy: 8-way data-parallel over the 4096 tokens (512 tokens/core),
weights replicated. No collectives. Everything is laid out
[feature, token] (transposed host-side) so every matmul has the
contraction dim on partitions and needs no on-device transposes.

All matmuls are fp8-e4m3 DoubleRow (256-deep contraction per MM at the
same 216 ns as a 128-deep bf16 MM => 2x MAC throughput), except the
bf16 tail of the down projection:

  phase 2: gate/up projections + SwiGLU, numerically exact via a
      residual split of x: x = x8 + r8 (both e4m3; r8 carries the
      e4m3 rounding error of x8, so x-hat is good to ~0.08%).
      Per k-tile-pair kp and 128-col block c:
        psum[c, T] += w8[kp].T @ x8[kp]   (DoubleRow, 256-deep)
        psum[c, T] += w8[kp].T @ r8[kp]   (same weights, r8 pass)
      Same MM count as a bf16 phase 2 (break-even compute) but HALF
      the weight bytes (fp8), which shortens the DMA-ramp-bound
      lead-in. inter = silu(gate*gs) * up  (kept resident).
  phase 3: down projection
      first IF8 i-tiles of inter stored e4m3, contracted as DoubleRow
      pairs (2x throughput, ~2.65% noise on that fraction -> the error
      budget); remaining i-tiles stored bf16, contracted bf16:
      psum[h_tile, T] += wd[i_tile].T @ inter[i_tile]

Ternary weights are exact in e4m3 (and the 2^-4 up-prescale keeps the
fp8-stored inter inside e4m3's +-240). N=512 = one PSUM bank. Weights
stream from HBM in blocks, alternating between the two HWDGE rings
(sync + scalar). ~40 warm-up matmuls on a memset tile run during the
DMA lead-in so the PE's HAM clock-gate reaches 8/8 before the first
real matmul.

Error budget (hard gate 2e-2): e4m3 eps ~= 2.65%; only the IF8/86
fraction of the down contraction is noisy -> rel err ~= 2.65% *
sqrt(46/86) ~= 1.95e-2 (hw-measured 1.952e-2). Budget spent entirely
in the down-proj: silu amplifies gate-path noise (a_g = 1.205) and the
up-path trade is exactly break-even, so no better allocation exists.
"""

import sys

for _p in ("/opt/trn_rl_repo",):
    if _p not in sys.path:
        sys.path.append(_p)

import numpy as np
import ml_dtypes

import concourse.bacc as bacc
import concourse.mybir as mybir
import concourse.tile as tile
from concourse.bass_utils import run_bass_kernel_spmd

BF16 = ml_dtypes.bfloat16
FP8 = ml_dtypes.float8_e4m3   # TRN FP8_EXP4: IEEE-style e4m3, max +-240

N_CORES = 8
H = 4096          # hidden
I = 11008         # intermediate
T = 512           # tokens per core (4096 / 8)
P = 128
KO = H // P       # 32 k-tiles for gate/up contraction
KP = KO // 2      # 16 k-tile pairs (DoubleRow)
IT = I // P       # 86 i-tiles
NG2 = IT // 2     # 43 phase-2 groups (2 i-tiles each)

IF8 = 46          # i-tiles of inter kept in fp8 (DoubleRow down-proj)
NP8 = IF8 // 2    # fp8 contraction pairs (23)
BLK8 = [12, 11]   # fp8 weight DMA blocks (pairs per block)
IT16 = IT - IF8   # 40 bf16 i-tiles
BLK16 = [16, 16, IT16 - 32]       # bf16 i-blocks [16,16,8]
HG = (H // P) // 4  # 8 phase-3 groups (4 h-tiles each)
N_WARM = 40       # PE warm-up matmuls (N=128 each) during DMA lead-in

DR = mybir.MatmulPerfMode.DoubleRow

_CACHE = {}


def _build_nc(gate_scale: float, up_scale: float, down_scale: float):
    nc = bacc.Bacc("TRN2", target_bir_lowering=False, debug=False,
                   enable_asserts=False, num_devices=N_CORES)
    f32 = mybir.dt.float32
    bf16 = mybir.dt.bfloat16
    f8 = mybir.dt.float8e4

    x8_d = nc.dram_tensor("x8", [P, KP, 2, T], f8, kind="ExternalInput")
    r8_d = nc.dram_tensor("r8", [P, KP, 2, T], f8, kind="ExternalInput")
    wgu_d = nc.dram_tensor("wgu", [NG2, P, KP, 2, 512], f8, kind="ExternalInput")
    wd8_d = nc.dram_tensor("wd8", [HG, P, NP8, 2, 512], f8, kind="ExternalInput")
    wd16_d = nc.dram_tensor("wd16", [HG, len(BLK16), P, 16, 512], bf16,
                            kind="ExternalInput")
    out_d = nc.dram_tensor("out", [HG, P, 4, T], f32, kind="ExternalOutput")

    with tile.TileContext(nc) as tc:
        with (
            tc.tile_pool(name="xpool", bufs=1) as xpool,
            tc.tile_pool(name="wpool", bufs=3) as wpool,
            tc.tile_pool(name="w8pool", bufs=2) as w8pool,
            tc.tile_pool(name="ipool", bufs=1) as ipool,
            tc.tile_pool(name="tpool", bufs=2) as tpool,
            tc.tile_pool(name="opool", bufs=2) as opool,
            tc.tile_pool(name="dpool", bufs=1) as dpool,
            tc.tile_pool(name="psum", bufs=2, space="PSUM") as psum,
        ):
            # ---- phase 0: PE warm-up ----
            # The HAM clock gate keeps the PE at 1.2 GHz until it has seen
            # ~3.4us of sustained matmul activity.  Spend the DMA lead-in
            # warming it on a memset tile so real matmuls run at 2.4 GHz
            # from the first one.
            dummy = dpool.tile([P, P], bf16)
            nc.vector.memset(dummy[:], 0)
            warm_ps = psum.tile([P, T], f32, tag="p0")
            for _ in range(N_WARM):
                nc.tensor.matmul(warm_ps[:, 0:P], dummy[:], dummy[:],
                                 start=True, stop=True)

            # x8 then r8's first half on the scalar HWDGE ring (parallel to
            # weight traffic on the sync ring), in escalating kp-slice chunks
            # so the first matmuls don't wait for the whole 2 MiB.  r8 is not
            # consumed until the r8 pass of group 0 (~64 MMs in); its tail
            # half rides the sync ring (idle between g0 and g1 weights).
            x8_sb = xpool.tile([P, KP, 2, T], f8, tag="x8")
            r8_sb = xpool.tile([P, KP, 2, T], f8, tag="r8")
            for lo, hi in ((0, 1), (1, 2), (2, 3), (3, 4), (4, 5), (5, 6),
                           (6, 8), (8, 10), (10, 12), (12, 14), (14, 16)):
                nc.scalar.dma_start(x8_sb[:, lo:hi], x8_d.ap()[:, lo:hi])
            for lo, hi in ((0, 2), (2, 4), (4, 6), (6, 8)):
                nc.scalar.dma_start(r8_sb[:, lo:hi], r8_d.ap()[:, lo:hi])
            inter8_sb = ipool.tile([P, IF8, T], f8, tag="i8")
            inter16_sb = ipool.tile([P, IT16, T], bf16, tag="i16")

            # g0's weights lead the sync ring in escalating chunks so MM 0
            # starts early; r8's tail follows them (needed ~14us after MM 0,
            # once g0's r8 pass reaches kp=8).
            w_g0 = wpool.tile([P, KP, 2, 512], f8, tag="w")
            for lo, hi in ((0, 1), (1, 2), (2, 3), (3, 4), (4, 6),
                           (6, 8), (8, 12), (12, 16)):
                nc.sync.dma_start(w_g0[:, lo:hi], wgu_d.ap()[0, :, lo:hi])
            for lo, hi in ((8, 10), (10, 12), (12, 14), (14, 16)):
                nc.sync.dma_start(r8_sb[:, lo:hi], r8_d.ap()[:, lo:hi])

            # ---- phase 2: gate/up DoubleRow matmuls + SwiGLU ----
            for g in range(NG2):
                pg0 = psum.tile([P, T], f32, tag="p0")
                pg1 = psum.tile([P, T], f32, tag="p1")
                pu0 = psum.tile([P, T], f32, tag="p2")
                pu1 = psum.tile([P, T], f32, tag="p3")
                if g == 0:
                    w = w_g0
                else:
                    w = wpool.tile([P, KP, 2, 512], f8, tag="w")
                    if g == 1:
                        # g1's weights ride the scalar ring (sync carries
                        # g0 + the r8 tail during the ramp window)
                        for lo, hi in ((0, 4), (4, 8), (8, 12), (12, 16)):
                            nc.scalar.dma_start(w[:, lo:hi], wgu_d.ap()[g, :, lo:hi])
                    else:
                        # alternate HWDGE rings
                        eng = nc.sync if g % 2 == 0 else nc.scalar
                        eng.dma_start(w[:], wgu_d.ap()[g])
                for xp, (st, sp) in ((x8_sb, (True, False)), (r8_sb, (False, True))):
                    for kp in range(KP):
                        rhs = xp[:, kp]
                        s0 = st and kp == 0
                        s1 = sp and kp == KP - 1
                        nc.tensor.matmul(pg0[:], w[:, kp, :, 0:128], rhs,
                                         start=s0, stop=s1, perf_mode=DR)
                        nc.tensor.matmul(pg1[:], w[:, kp, :, 128:256], rhs,
                                         start=s0, stop=s1, perf_mode=DR)
                        nc.tensor.matmul(pu0[:], w[:, kp, :, 256:384], rhs,
                                         start=s0, stop=s1, perf_mode=DR)
                        nc.tensor.matmul(pu1[:], w[:, kp, :, 384:512], rhs,
                                         start=s0, stop=s1, perf_mode=DR)
                for j, (pg, pu) in enumerate(((pg0, pu0), (pg1, pu1))):
                    it = 2 * g + j
                    silu_t = tpool.tile([P, T], bf16, tag="silu")
                    nc.scalar.activation(silu_t[:], pg[:],
                                         mybir.ActivationFunctionType.Silu,
                                         scale=gate_scale)
                    dst = (inter8_sb[:, it, :] if it < IF8
                           else inter16_sb[:, it - IF8, :])
                    nc.vector.tensor_mul(dst, silu_t[:], pu[:])

            # ---- phase 3: down matmul ----
            # up-weights carry an exact 2^-4 factor (host-side) so the stored
            # inter = silu(g)*u_raw/16 stays ~[-53, 53], inside e4m3's +-240
            out_scale = up_scale * down_scale * 16.0
            for hg in range(HG):
                pd = [psum.tile([P, T], f32, tag=f"p{j}", name=f"pd{j}") for j in range(4)]
                # fp8 DoubleRow part: i-tiles 0..IF8-1, two tiles per MM
                p0 = 0
                for blk, bs in enumerate(BLK8):
                    w8 = w8pool.tile([P, max(BLK8), 2, 512], f8, tag="w8")
                    eng = nc.sync if blk == 0 else nc.scalar
                    eng.dma_start(w8[:, 0:bs], wd8_d.ap()[hg, :, p0:p0 + bs])
                    for mb in range(bs):
                        pair = p0 + mb
                        rhs = inter8_sb[:, 2 * pair:2 * pair + 2, :]
                        st = pair == 0
                        for j in range(4):
                            nc.tensor.matmul(pd[j][:], w8[:, mb, :, j * 128:(j + 1) * 128],
                                             rhs, start=st, stop=False, perf_mode=DR)
                    p0 += bs
                # bf16 part: i-tiles IF8..IT-1
                for b, bs in enumerate(BLK16):
                    wd_sb = wpool.tile([P, 16, 512], bf16, tag="w")
                    eng = nc.sync if b % 2 == 0 else nc.scalar
                    eng.dma_start(wd_sb[:, 0:bs, :], wd16_d.ap()[hg, b, :, 0:bs, :])
                    # very last block of the kernel: finish the four psum
                    # chains j-sequentially so pd0 stops ~1.3us before pd3;
                    # each epilogue + out-DMA drains while later chains compute
                    stagger = (hg == HG - 1) and (b == len(BLK16) - 1)
                    if stagger:
                        for j in range(4):
                            for io in range(bs):
                                i16 = b * 16 + io
                                nc.tensor.matmul(pd[j][:],
                                                 wd_sb[:, io, j * 128:(j + 1) * 128],
                                                 inter16_sb[:, i16, :],
                                                 start=False, stop=(i16 == IT16 - 1))
                    else:
                        for io in range(bs):
                            i16 = b * 16 + io
                            rhs = inter16_sb[:, i16, :]
                            sp = i16 == IT16 - 1
                            for j in range(4):
                                nc.tensor.matmul(pd[j][:], wd_sb[:, io, j * 128:(j + 1) * 128],
                                                 rhs, start=False, stop=sp)
                ob = opool.tile([P, 4, T], f32, tag="ob")
                for j in range(4):
                    # alternate ACT/DVE (and the two HWDGE rings) so the final
                    # scale-copies and out-DMAs don't serialize on one queue
                    if j % 2 == 0:
                        nc.scalar.activation(ob[:, j, :], pd[j][:],
                                             mybir.ActivationFunctionType.Copy,
                                             scale=out_scale)
                        nc.sync.dma_start(out_d.ap()[hg, :, j, :], ob[:, j, :])
                    elif hg == HG - 1 and j == 3:
                        # very last output: DVE and ACT scale one half each in
                        # parallel; the half-DMAs ride separate rings
                        nc.vector.tensor_scalar_mul(ob[:, j, 0:256],
                                                    pd[j][:, 0:256], out_scale)
                        nc.scalar.dma_start(out_d.ap()[hg, :, j, 0:256],
                                            ob[:, j, 0:256])
                        nc.scalar.activation(ob[:, j, 256:512],
                                             pd[j][:, 256:512],
                                             mybir.ActivationFunctionType.Copy,
                                             scale=out_scale)
                        nc.sync.dma_start(out_d.ap()[hg, :, j, 256:512],
                                          ob[:, j, 256:512])
                    else:
                        nc.vector.tensor_scalar_mul(ob[:, j, :], pd[j][:], out_scale)
                        nc.scalar.dma_start(out_d.ap()[hg, :, j, :], ob[:, j, :])

    nc.compile()
    return nc


def _pack_weights(gate_w, up_w, down_w):
    # wgu[g, p, kp, e, c]: c 0:128 -> gate i-tile 2g, 128:256 -> gate 2g+1,
    # 256:384 -> up 2g, 384:512 -> up 2g+1 (up carries exact 2^-4).
    # element = W[row(c), k=(kp*2+e)*128+p]
    Gr = np.ascontiguousarray(gate_w, dtype=np.float32).reshape(NG2, 256, H)
    Ur = np.ascontiguousarray(up_w, dtype=np.float32).reshape(NG2, 256, H) * (2.0 ** -4)
    W4 = np.concatenate([Gr, Ur], axis=1)                   # [g, 512, H]
    wgu = W4.reshape(NG2, 512, KP, 2, P).transpose(0, 4, 2, 3, 1)  # [g,p,kp,e,c]
    wgu = np.ascontiguousarray(wgu.astype(FP8))

    dw = np.ascontiguousarray(down_w, dtype=np.float32)     # [H, I]

    # wd8[hg, p, pair, e, j*128+hp] = dw[(hg*4+j)*128+hp, (pair*2+e)*128+p]
    d8 = dw[:, :IF8 * P].reshape(HG, 4, P, NP8, 2, P)       # [hg, j, hp, pair, e, p]
    wd8 = d8.transpose(0, 5, 3, 4, 1, 2)                    # [hg, p, pair, e, j, hp]
    wd8 = np.ascontiguousarray(wd8.reshape(HG, P, NP8, 2, 512).astype(FP8))

    # wd16[hg, b, p, io, j*128+hp] = dw[(hg*4+j)*128+hp, (IF8+b*16+io)*128+p]
    dp = np.zeros((H, len(BLK16) * 16 * P), dtype=np.float32)
    dp[:, :IT16 * P] = dw[:, IF8 * P:]
    d16 = dp.reshape(HG, 4, P, len(BLK16), 16, P)           # [hg, j, hp, b, io, p]
    wd16 = d16.transpose(0, 3, 5, 4, 1, 2)                  # [hg, b, p, io, j, hp]
    wd16 = np.ascontiguousarray(wd16.reshape(HG, len(BLK16), P, 16, 512).astype(BF16))
    return wgu, wd8, wd16


def _pack_x(x):
    # x = x8 + r8 residual split, both e4m3, laid out [p, kp, e, t]
    tokens = np.ascontiguousarray(x, dtype=np.float32).reshape(N_CORES * T, H)
    packs = []
    for c in range(N_CORES):
        xs = tokens[c * T:(c + 1) * T]                       # [T, H]
        xt = xs.T.reshape(KP, 2, P, T).transpose(2, 0, 1, 3)  # [p, kp, e, t]
        x8 = xt.astype(FP8)
        r8 = (xt - x8.astype(np.float32)).astype(FP8)
        packs.append((np.ascontiguousarray(x8), np.ascontiguousarray(r8)))
    return packs


def _unpack_out(res_list, out_dtype):
    outs = []
    for c in range(N_CORES):
        a = res_list[c]["out"]                                # [HG, P, 4, T] f32
        ht = a.transpose(0, 2, 1, 3).reshape(H, T)            # [h, t]
        outs.append(ht.T)                                     # [t, h]
    full = np.concatenate(outs, axis=0)                       # [4096, H]
    return full.reshape(2, N_CORES * T // 2, H).astype(out_dtype, copy=False)


def _run(x, gate_w, up_w, down_w, gate_scale, up_scale, down_scale,
         trace=False, **run_kwargs):
    key = (float(gate_scale), float(up_scale), float(down_scale))
    if key not in _CACHE:
        _CACHE.clear()
        _CACHE[key] = _build_nc(*key)
    nc = _CACHE[key]

    wgu, wd8, wd16 = _pack_weights(gate_w, up_w, down_w)
    packs = _pack_x(x)
    in_maps = [{"x8": packs[c][0], "r8": packs[c][1], "wgu": wgu,
                "wd8": wd8, "wd16": wd16}
               for c in range(N_CORES)]
    try:
        res = run_bass_kernel_spmd(nc, in_maps, core_ids=list(range(N_CORES)),
                                   trace=trace, **run_kwargs)
    except Exception:
        # transient device/runtime hiccups: one retry
        res = run_bass_kernel_spmd(nc, in_maps, core_ids=list(range(N_CORES)),
                                   trace=trace, **run_kwargs)
    out = _unpack_out(res.results, np.asarray(x).dtype)
    return out, res


def kernel(x, gate_w, up_w, down_w, gate_scale, up_scale, down_scale):
    out, _ = _run(x, gate_w, up_w, down_w, gate_scale, up_scale, down_scale)
    return out


# revision 23
# speedup vs baseline: 1.0016x; 1.0016x over previous
"""BitNet MLP (SwiGLU, ternary weights) on 8 Trainium2 NeuronCores.

Strategy: 8-way data-parallel over the 4096 tokens (512 tokens/core),
weights replicated. No collectives. Everything is laid out
[feature, token] (transposed host-side) so every matmul has the
contraction dim on partitions and needs no on-device transposes.

All matmuls are fp8-e4m3 DoubleRow (256-deep contraction per MM at the
same 216 ns as a 128-deep bf16 MM => 2x MAC throughput), except the
bf16 tail of the down projection:

  phase 2: gate/up projections + SwiGLU, numerically exact via a
      residual split of x: x = x8 + r8 (both e4m3; r8 carries the
      e4m3 rounding error of x8, so x-hat is good to ~0.08%).
      Per k-tile-pair kp and 128-col block c:
        psum[c, T] += w8[kp].T @ x8[kp]   (DoubleRow, 256-deep)
        psum[c, T] += w8[kp].T @ r8[kp]   (same weights, r8 pass)
      Same MM count as a bf16 phase 2 (break-even compute) but HALF
      the weight bytes (fp8), which shortens the DMA-ramp-bound
      lead-in. inter = silu(gate*gs) * up  (kept resident).
  phase 3: down projection
      first IF8 i-tiles of inter stored e4m3, contracted as DoubleRow
      pairs (2x throughput, ~2.65% noise on that fraction -> the error
      budget); remaining i-tiles stored bf16, contracted bf16:
      psum[h_tile, T] += wd[i_tile].T @ inter[i_tile]

Ternary weights are exact in e4m3 (and the 2^-4 up-prescale keeps the
fp8-stored inter inside e4m3's +-240). N=512 = one PSUM bank. Weights
stream from HBM in blocks, alternating between the two HWDGE rings
(sync + scalar). ~40 warm-up matmuls on a memset tile run during the
DMA lead-in so the PE's HAM clock-gate reaches 8/8 before the first
real matmul.

Error budget (hard gate 2e-2): e4m3 eps ~= 2.65%; only the IF8/86
fraction of the down contraction is noisy -> rel err ~= 2.65% *
sqrt(46/86) ~= 1.95e-2 (hw-measured 1.952e-2). Budget spent entirely
in the down-proj: silu amplifies gate-path noise (a_g = 1.205) and the
up-path trade is exactly break-even, so no better allocation exists.
"""

import sys

for _p in ("/opt/trn_rl_repo",):
    if _p not in sys.path:
        sys.path.append(_p)

import numpy as np
import ml_dtypes

import concourse.bacc as bacc
import concourse.mybir as mybir
import concourse.tile as tile
from concourse.bass_utils import run_bass_kernel_spmd

BF16 = ml_dtypes.bfloat16
FP8 = ml_dtypes.float8_e4m3   # TRN FP8_EXP4: IEEE-style e4m3, max +-240

N_CORES = 8
H = 4096          # hidden
I = 11008         # intermediate
T = 512           # tokens per core (4096 / 8)
P = 128
KO = H // P       # 32 k-tiles for gate/up contraction
KP = KO // 2      # 16 k-tile pairs (DoubleRow)
IT = I // P       # 86 i-tiles
NG2 = IT // 2     # 43 phase-2 groups (2 i-tiles each)

IF8 = 46          # i-tiles of inter kept in fp8 (DoubleRow down-proj)
NP8 = IF8 // 2    # fp8 contraction pairs (23)
BLK8 = [12, 11]   # fp8 weight DMA blocks (pairs per block)
IT16 = IT - IF8   # 40 bf16 i-tiles
BLK16 = [16, 16, IT16 - 32]       # bf16 i-blocks [16,16,8]
HG = (H // P) // 4  # 8 phase-3 groups (4 h-tiles each)
N_WARM = 40       # PE warm-up matmuls (N=128 each) during DMA lead-in

DR = mybir.MatmulPerfMode.DoubleRow

_CACHE = {}


def _build_nc(gate_scale: float, up_scale: float, down_scale: float):
    nc = bacc.Bacc("TRN2", target_bir_lowering=False, debug=False,
                   enable_asserts=False, num_devices=N_CORES)
    f32 = mybir.dt.float32
    bf16 = mybir.dt.bfloat16
    f8 = mybir.dt.float8e4

    x8_d = nc.dram_tensor("x8", [P, KP, 2, T], f8, kind="ExternalInput")
    r8_d = nc.dram_tensor("r8", [P, KP, 2, T], f8, kind="ExternalInput")
    wgu_d = nc.dram_tensor("wgu", [NG2, P, KP, 2, 512], f8, kind="ExternalInput")
    wd8_d = nc.dram_tensor("wd8", [HG, P, NP8, 2, 512], f8, kind="ExternalInput")
    wd16_d = nc.dram_tensor("wd16", [HG, len(BLK16), P, 16, 512], bf16,
                            kind="ExternalInput")
    out_d = nc.dram_tensor("out", [HG, P, 4, T], f32, kind="ExternalOutput")

    with tile.TileContext(nc) as tc:
        with (
            tc.tile_pool(name="xpool", bufs=1) as xpool,
            tc.tile_pool(name="wpool", bufs=3) as wpool,
            tc.tile_pool(name="w8pool", bufs=2) as w8pool,
            tc.tile_pool(name="ipool", bufs=1) as ipool,
            tc.tile_pool(name="tpool", bufs=2) as tpool,
            tc.tile_pool(name="opool", bufs=2) as opool,
            tc.tile_pool(name="dpool", bufs=1) as dpool,
            tc.tile_pool(name="psum", bufs=2, space="PSUM") as psum,
        ):
            # ---- phase 0: PE warm-up ----
            # The HAM clock gate keeps the PE at 1.2 GHz until it has seen
            # ~3.4us of sustained matmul activity.  Spend the DMA lead-in
            # warming it on a memset tile so real matmuls run at 2.4 GHz
            # from the first one.
            dummy = dpool.tile([P, P], bf16)
            nc.vector.memset(dummy[:], 0)
            warm_ps = psum.tile([P, T], f32, tag="p0")
            for _ in range(N_WARM):
                nc.tensor.matmul(warm_ps[:, 0:P], dummy[:], dummy[:],
                                 start=True, stop=True)

            # x8 then r8's first half on the scalar HWDGE ring (parallel to
            # weight traffic on the sync ring), in escalating kp-slice chunks
            # so the first matmuls don't wait for the whole 2 MiB.  r8 is not
            # consumed until the r8 pass of group 0 (~64 MMs in); its tail
            # half rides the sync ring (idle between g0 and g1 weights).
            x8_sb = xpool.tile([P, KP, 2, T], f8, tag="x8")
            r8_sb = xpool.tile([P, KP, 2, T], f8, tag="r8")
            for lo, hi in ((0, 1), (1, 2), (2, 3), (3, 4), (4, 5), (5, 6),
                           (6, 8), (8, 10), (10, 12), (12, 14), (14, 16)):
                nc.scalar.dma_start(x8_sb[:, lo:hi], x8_d.ap()[:, lo:hi])
            for lo, hi in ((0, 2), (2, 4), (4, 6), (6, 8)):
                nc.scalar.dma_start(r8_sb[:, lo:hi], r8_d.ap()[:, lo:hi])
            inter8_sb = ipool.tile([P, IF8, T], f8, tag="i8")
            inter16_sb = ipool.tile([P, IT16, T], bf16, tag="i16")

            # g0's weights lead the sync ring in escalating chunks so MM 0
            # starts early; r8's tail follows them (needed ~14us after MM 0,
            # once g0's r8 pass reaches kp=8).
            w_g0 = wpool.tile([P, KP, 2, 512], f8, tag="w")
            for lo, hi in ((0, 1), (1, 2), (2, 3), (3, 4), (4, 6),
                           (6, 8), (8, 12), (12, 16)):
                nc.sync.dma_start(w_g0[:, lo:hi], wgu_d.ap()[0, :, lo:hi])
            for lo, hi in ((8, 10), (10, 12), (12, 14), (14, 16)):
                nc.sync.dma_start(r8_sb[:, lo:hi], r8_d.ap()[:, lo:hi])

            # ---- phase 2: gate/up DoubleRow matmuls + SwiGLU ----
            for g in range(NG2):
                pg0 = psum.tile([P, T], f32, tag="p0")
                pg1 = psum.tile([P, T], f32, tag="p1")
                pu0 = psum.tile([P, T], f32, tag="p2")
                pu1 = psum.tile([P, T], f32, tag="p3")
                if g == 0:
                    w = w_g0
                else:
                    w = wpool.tile([P, KP, 2, 512], f8, tag="w")
                    if g == 1:
                        # g1's weights ride the scalar ring (sync carries
                        # g0 + the r8 tail during the ramp window)
                        for lo, hi in ((0, 4), (4, 8), (8, 12), (12, 16)):
                            nc.scalar.dma_start(w[:, lo:hi], wgu_d.ap()[g, :, lo:hi])
                    else:
                        # alternate HWDGE rings
                        eng = nc.sync if g % 2 == 0 else nc.scalar
                        eng.dma_start(w[:], wgu_d.ap()[g])
                for xp, (st, sp) in ((x8_sb, (True, False)), (r8_sb, (False, True))):
                    for kp in range(KP):
                        rhs = xp[:, kp]
                        s0 = st and kp == 0
                        s1 = sp and kp == KP - 1
                        nc.tensor.matmul(pg0[:], w[:, kp, :, 0:128], rhs,
                                         start=s0, stop=s1, perf_mode=DR)
                        nc.tensor.matmul(pg1[:], w[:, kp, :, 128:256], rhs,
                                         start=s0, stop=s1, perf_mode=DR)
                        nc.tensor.matmul(pu0[:], w[:, kp, :, 256:384], rhs,
                                         start=s0, stop=s1, perf_mode=DR)
                        nc.tensor.matmul(pu1[:], w[:, kp, :, 384:512], rhs,
                                         start=s0, stop=s1, perf_mode=DR)
                for j, (pg, pu) in enumerate(((pg0, pu0), (pg1, pu1))):
                    it = 2 * g + j
                    silu_t = tpool.tile([P, T], bf16, tag="silu")
                    nc.scalar.activation(silu_t[:], pg[:],
                                         mybir.ActivationFunctionType.Silu,
                                         scale=gate_scale)
                    dst = (inter8_sb[:, it, :] if it < IF8
                           else inter16_sb[:, it - IF8, :])
                    nc.vector.tensor_mul(dst, silu_t[:], pu[:])

            # ---- phase 3: down matmul ----
            # up-weights carry an exact 2^-4 factor (host-side) so the stored
            # inter = silu(g)*u_raw/16 stays ~[-53, 53], inside e4m3's +-240
            out_scale = up_scale * down_scale * 16.0
            for hg in range(HG):
                pd = [psum.tile([P, T], f32, tag=f"p{j}", name=f"pd{j}") for j in range(4)]
                # fp8 DoubleRow part: i-tiles 0..IF8-1, two tiles per MM
                p0 = 0
                for blk, bs in enumerate(BLK8):
                    w8 = w8pool.tile([P, max(BLK8), 2, 512], f8, tag="w8")
                    eng = nc.sync if blk == 0 else nc.scalar
                    eng.dma_start(w8[:, 0:bs], wd8_d.ap()[hg, :, p0:p0 + bs])
                    for mb in range(bs):
                        pair = p0 + mb
                        rhs = inter8_sb[:, 2 * pair:2 * pair + 2, :]
                        st = pair == 0
                        for j in range(4):
                            nc.tensor.matmul(pd[j][:], w8[:, mb, :, j * 128:(j + 1) * 128],
                                             rhs, start=st, stop=False, perf_mode=DR)
                    p0 += bs
                # bf16 part: i-tiles IF8..IT-1
                for b, bs in enumerate(BLK16):
                    wd_sb = wpool.tile([P, 16, 512], bf16, tag="w")
                    eng = nc.sync if b % 2 == 0 else nc.scalar
                    eng.dma_start(wd_sb[:, 0:bs, :], wd16_d.ap()[hg, b, :, 0:bs, :])
                    # very last block of the kernel: finish the four psum
                    # chains j-sequentially so pd0 stops ~1.3us before pd3;
                    # each epilogue + out-DMA drains while later chains compute
                    stagger = (hg == HG - 1) and (b == len(BLK16) - 1)
                    if stagger:
                        for j in range(4):
                            for io in range(bs):
                                i16 = b * 16 + io
                                nc.tensor.matmul(pd[j][:],
                                                 wd_sb[:, io, j * 128:(j + 1) * 128],
                                                 inter16_sb[:, i16, :],
                                                 start=False, stop=(i16 == IT16 - 1))
                    else:
                        for io in range(bs):
                            i16 = b * 16 + io
                            rhs = inter16_sb[:, i16, :]
                            sp = i16 == IT16 - 1
                            for j in range(4):
                                nc.tensor.matmul(pd[j][:], wd_sb[:, io, j * 128:(j + 1) * 128],
                                                 rhs, start=False, stop=sp)
                ob = opool.tile([P, 4, T], f32, tag="ob")
                for j in range(4):
                    # alternate ACT/DVE (and the two HWDGE rings) so the final
                    # scale-copies and out-DMAs don't serialize on one queue
                    if j % 2 == 0:
                        nc.scalar.activation(ob[:, j, :], pd[j][:],
                                             mybir.ActivationFunctionType.Copy,
                                             scale=out_scale)
                        nc.sync.dma_start(out_d.ap()[hg, :, j, :], ob[:, j, :])
                    elif hg == HG - 1 and j == 3:
                        # very last output: chunk the epilogue so the first
                        # half's DMA overlaps the second half's scale-copy
                        nc.vector.tensor_scalar_mul(ob[:, j, 0:256],
                                                    pd[j][:, 0:256], out_scale)
                        nc.scalar.dma_start(out_d.ap()[hg, :, j, 0:256],
                                            ob[:, j, 0:256])
                        nc.vector.tensor_scalar_mul(ob[:, j, 256:512],
                                                    pd[j][:, 256:512], out_scale)
                        nc.sync.dma_start(out_d.ap()[hg, :, j, 256:512],
                                          ob[:, j, 256:512])
                    else:
                        nc.vector.tensor_scalar_mul(ob[:, j, :], pd[j][:], out_scale)
                        nc.scalar.dma_start(out_d.ap()[hg, :, j, :], ob[:, j, :])

    nc.compile()
    return nc


def _pack_weights(gate_w, up_w, down_w):
    # wgu[g, p, kp, e, c]: c 0:128 -> gate i-tile 2g, 128:256 -> gate 2g+1,
    # 256:384 -> up 2g, 384:512 -> up 2g+1 (up carries exact 2^-4).
    # element = W[row(c), k=(kp*2+e)*128+p]
    Gr = np.ascontiguousarray(gate_w, dtype=np.float32).reshape(NG2, 256, H)
    Ur = np.ascontiguousarray(up_w, dtype=np.float32).reshape(NG2, 256, H) * (2.0 ** -4)
    W4 = np.concatenate([Gr, Ur], axis=1)                   # [g, 512, H]
    wgu = W4.reshape(NG2, 512, KP, 2, P).transpose(0, 4, 2, 3, 1)  # [g,p,kp,e,c]
    wgu = np.ascontiguousarray(wgu.astype(FP8))

    dw = np.ascontiguousarray(down_w, dtype=np.float32)     # [H, I]

    # wd8[hg, p, pair, e, j*128+hp] = dw[(hg*4+j)*128+hp, (pair*2+e)*128+p]
    d8 = dw[:, :IF8 * P].reshape(HG, 4, P, NP8, 2, P)       # [hg, j, hp, pair, e, p]
    wd8 = d8.transpose(0, 5, 3, 4, 1, 2)                    # [hg, p, pair, e, j, hp]
    wd8 = np.ascontiguousarray(wd8.reshape(HG, P, NP8, 2, 512).astype(FP8))

    # wd16[hg, b, p, io, j*128+hp] = dw[(hg*4+j)*128+hp, (IF8+b*16+io)*128+p]
    dp = np.zeros((H, len(BLK16) * 16 * P), dtype=np.float32)
    dp[:, :IT16 * P] = dw[:, IF8 * P:]
    d16 = dp.reshape(HG, 4, P, len(BLK16), 16, P)           # [hg, j, hp, b, io, p]
    wd16 = d16.transpose(0, 3, 5, 4, 1, 2)                  # [hg, b, p, io, j, hp]
    wd16 = np.ascontiguousarray(wd16.reshape(HG, len(BLK16), P, 16, 512).astype(BF16))
    return wgu, wd8, wd16


def _pack_x(x):
    # x = x8 + r8 residual split, both e4m3, laid out [p, kp, e, t]
    tokens = np.ascontiguousarray(x, dtype=np.float32).reshape(N_CORES * T, H)
    packs = []
    for c in range(N_CORES):
        xs = tokens[c * T:(c + 1) * T]                       # [T, H]
        xt = xs.T.reshape(KP, 2, P, T).transpose(2, 0, 1, 3)  # [p, kp, e, t]
        x8 = xt.astype(FP8)
        r8 = (xt - x8.astype(np.float32)).astype(FP8)
        packs.append((np.ascontiguousarray(x8), np.ascontiguousarray(r8)))
    return packs


def _unpack_out(res_list, out_dtype):
    outs = []
    for c in range(N_CORES):
        a = res_list[c]["out"]                                # [HG, P, 4, T] f32
        ht = a.transpose(0, 2, 1, 3).reshape(H, T)            # [h, t]
        outs.append(ht.T)                                     # [t, h]
    full = np.concatenate(outs, axis=0)                       # [4096, H]
    return full.reshape(2, N_CORES * T // 2, H).astype(out_dtype, copy=False)


def _run(x, gate_w, up_w, down_w, gate_scale, up_scale, down_scale,
         trace=False, **run_kwargs):
    key = (float(gate_scale), float(up_scale), float(down_scale))
    if key not in _CACHE:
        _CACHE.clear()
        _CACHE[key] = _build_nc(*key)
    nc = _CACHE[key]

    wgu, wd8, wd16 = _pack_weights(gate_w, up_w, down_w)
    packs = _pack_x(x)
    in_maps = [{"x8": packs[c][0], "r8": packs[c][1], "wgu": wgu,
                "wd8": wd8, "wd16": wd16}
               for c in range(N_CORES)]
    try:
        res = run_bass_kernel_spmd(nc, in_maps, core_ids=list(range(N_CORES)),
                                   trace=trace, **run_kwargs)
    except Exception:
        # transient device/runtime hiccups: one retry
        res = run_bass_kernel_spmd(nc, in_maps, core_ids=list(range(N_CORES)),
                                   trace=trace, **run_kwargs)
    out = _unpack_out(res.results, np.asarray(x).dtype)
    return out, res


def kernel(x, gate_w, up_w, down_w, gate_scale, up_scale, down_scale):
    out, _ = _run(x, gate_w, up_w, down_w, gate_scale, up_scale, down_scale)
    return out
